# revision 42
# baseline (speedup 1.0000x reference)
"""Trainium2 Bass kernel for nn_Attention_Temp_1468878815458.

Math: the reference computes
    pos   = arange(S) @ Wp.T + bp                       # (S,)
    embed = x.squeeze(1) + pos[:, None]                 # (B,S,D)
    v/k/q = embed @ {Wv,Wk,Wq}.T
    scores[b,x,y]  = (sum_q queries[b,q,x]) * (sum_k keys[b,k,y])
    attention      = softmax(scores, axis=1)            # over x
    out[b,v,y]     = sum_x attention[b,x,y] * sum_n values[b,v,n]

Since softmax normalizes over axis=1 and is then *summed* over axis=1,
sum_x attention[b,x,y] == 1 exactly.  Therefore
    out[b,s,y] = sum_n values[b,s,n]
               = (x[b,0,s,:] + pos[s]) . wv      for every y,
where wv[d] = sum_n Wv[n,d].  The kernel streams x once, computes the
per-row weighted sum with wv, adds the per-s bias pos[s]*sum(wv), and
broadcasts the scalar across the last dim.

Sharding: pure data parallel over batch, 1024 batches per core.  Each
core's shard is viewed as (128 partitions, 6144 f32): partition p holds
64 consecutive rows (8 batches x 8 seq) contiguously -> fully
contiguous DMA in AND out (24KB runs/partition).
"""

import numpy as np

import concourse.bass as bass
import concourse.mybir as mybir
from concourse.bass import broadcast_tensor_aps
from concourse.bass_utils import run_bass_kernel_spmd
from concourse.tile import TileContext

N_CORES = 8
B, S, D = 8192, 8, 96
BPC = B // N_CORES          # 1024 batches per core
ROWS = BPC * S              # 8192 rows of length D per core
P = 128                     # SBUF partitions
FREE = ROWS * D // P        # 6144 f32 per partition
RPP = ROWS // P             # 64 rows per partition
# pipeline chunk sizes in rows-per-partition: small chunks at the start
# (compute starts sooner) and end (shorter drain tail), big in the middle
# (fewer DMA triggers / per-op overheads).  GP_MUL marks chunks whose
# multiply runs on GPSIMD (otherwise idle) to shorten the DVE span.
CHUNK_ROWS = [4, 6, 10, 12, 12, 12, 6, 1, 1]
# last chunks run their whole tail (bias, broadcast) on DVE to avoid
# cross-engine hops after the final reduce
DVE_TAIL = 2
assert sum(CHUNK_ROWS) == RPP
NCH = len(CHUNK_ROWS)

_NC_CACHE = None


def _build() -> bass.Bass:
    # seq codegen lowers multi-wait sync (e.g. the kernel-tail drain) to
    # sequencer commands; this walrus build allows only 1 wait per inst
    nc = bass.Bass(use_seq_codegen=True, enable_partition_id=False)
    x = nc.declare_dram_parameter("x", [P, FREE], mybir.dt.float32, isOutput=False)
    # combined constants: [:, :D] = wv replicated, [:, D:D+RPP] = per-row bias
    wb = nc.declare_dram_parameter("wb", [P, D + RPP], mybir.dt.float32, isOutput=False)
    # wv again, pre-cast to bf16 (the x stream is cast f32->bf16 in-DMA,
    # which makes the DVE multiply eligible for the 2x perf mode)
    wvh = nc.declare_dram_parameter("wvh", [P, D], mybir.dt.bfloat16, isOutput=False)
    # bf16 output halves the out-stream HBM bytes; host upcasts to f32.
    # rowdot values are O(10); bf16 keeps rel err ~4e-3, well under budget
    out = nc.declare_dram_parameter("out", [P, FREE], mybir.dt.bfloat16, isOutput=True)

    with TileContext(nc) as tc:
        with (
            tc.tile_pool(name="const", bufs=1) as cpool,
            # unique tag per chunk -> each tile gets its own slot: no slot
            # reuse, no WAR waits
            tc.tile_pool(name="xp", bufs=1) as xpool,
            tc.tile_pool(name="pp", bufs=4) as ppool,
            tc.tile_pool(name="op", bufs=1) as opool,
            tc.tile_pool(name="rp", bufs=1) as rpool,
        ):
            wb_sb = cpool.tile([P, D + RPP], mybir.dt.float32)
            # issued first on the sync ring: completes long before any
            # consumer; its waits are absorbed by the NOP-split pass
            nc.sync.dma_start(out=wb_sb[:], in_=wb[:])
            bias_sb = wb_sb[:, D : D + RPP]
            wvh_sb = cpool.tile([P, D], mybir.dt.bfloat16)
            nc.sync.dma_start(out=wvh_sb[:], in_=wvh[:])

            r0 = 0
            ot = None
            ot_r0 = 0
            ot_fill = 0
            for c, chr_ in enumerate(CHUNK_ROWS):
                chf = chr_ * D
                f0 = r0 * D
                xt = xpool.tile([P, chf], mybir.dt.bfloat16, tag=f"xt{c}")
                # SWDGE: casts f32 -> bf16 in the DMA datapath
                nc.gpsimd.dma_start(out=xt[:], in_=x[:, f0 : f0 + chf])

                x3 = xt[:].rearrange("p (r d) -> p r d", d=D)
                wv3 = wvh_sb[:].rearrange("p (r d) -> p r d", r=1)
                _, wv3b = broadcast_tensor_aps(x3, wv3)
                pt = ppool.tile([P, chf], mybir.dt.bfloat16, tag="pt")
                p3 = pt[:, :chf].rearrange("p (r d) -> p r d", d=D)
                nc.vector.tensor_tensor(
                    out=p3, in0=x3, in1=wv3b, op=mybir.AluOpType.mult
                )
                # fold the 96-wide rows to 48 with a 2x-mode bf16 add, then
                # reduce 48 -> 1: ~35% less DVE time than reducing 96 wide
                h = D // 2
                lo = p3[:, :, :h]
                hi = p3[:, :, h:]
                nc.vector.tensor_tensor(
                    out=lo, in0=lo, in1=hi, op=mybir.AluOpType.add
                )

                rd = rpool.tile([P, chr_], mybir.dt.float32, tag=f"rd{c}")
                nc.vector.reduce_sum(out=rd[:], in_=lo, axis=mybir.AxisListType.X)
                tail = c >= NCH - DVE_TAIL
                # bias add on the otherwise-idle GPSIMD engine (DVE at tail)
                add_eng = nc.vector if tail else nc.gpsimd
                add_eng.tensor_add(
                    out=rd[:], in0=rd[:], in1=bias_sb[:, r0 : r0 + chr_]
                )

                # out tiles hold two consecutive chunks -> half the out-DMA
                # triggers on the ACT ring
                if ot is None:
                    pair_rows = chr_ + (CHUNK_ROWS[c + 1] if c + 1 < NCH else 0)
                    pair_free = pair_rows * D
                    ot = opool.tile([P, pair_free], mybir.dt.bfloat16, tag=f"ot{c}")
                    ot_r0 = r0
                    ot_fill = 0
                ot3 = ot[:, ot_fill : ot_fill + chf].rearrange(
                    "p (r d) -> p r d", d=D
                )
                rd3 = rd[:].rearrange("p (r d) -> p r d", d=1)
                _, rd3b = broadcast_tensor_aps(ot3, rd3)
                if tail:
                    nc.vector.tensor_copy(out=ot3, in_=rd3b)
                else:
                    nc.scalar.copy(out=ot3, in_=rd3b)
                ot_fill += chf
                r0 += chr_

                if ot_fill == pair_free or c + 1 == NCH:
                    # ACT ring: FIFO-clean (each out's broadcast precedes it
                    # in the ACT queue, so the trigger never stalls the ring)
                    nc.scalar.dma_start(
                        out=out[:, ot_r0 * D : ot_r0 * D + ot_fill],
                        in_=ot[:, :ot_fill],
                    )
                    ot = None
    _strip_unused_const_memsets(nc)
    _split_multi_waits(nc)
    return nc


def _strip_unused_const_memsets(nc: bass.Bass) -> None:
    """Bass unconditionally memsets 4 const SBUF tensors on GPSIMD in the
    preamble (~3us on the init-barrier critical path).  This kernel never
    reads them; drop the memsets.  The init all-engine barrier that
    followed them is also dead once they're gone: engines are independent
    until the Tile-emitted semaphores in the body, and NRT guarantees a
    clean sem state at NEFF start."""
    for f in nc.m.functions:
        for bb in f.blocks:
            if bb.name != "main":
                continue
            keep = []
            for inst in bb.instructions:
                if isinstance(
                    inst, mybir.InstMemset | mybir.InstDrain | mybir.InstEventSemaphore
                ):
                    continue
                keep.append(inst)
            if len(keep) != len(bb.instructions):
                bb.instructions[:] = keep


def _split_multi_waits(nc: bass.Bass) -> None:
    """Walrus (this build) allows only one sync wait per instruction.

    Tile's kernel-tail drain merges waits on every DMA lane + engine sem
    into one instruction; split the extras onto same-engine NOPs placed
    immediately before it.
    """
    for f in nc.m.functions:
        for bb in f.blocks:
            insts = bb.instructions
            i = 0
            while i < len(insts):
                inst = insts[i]
                si = inst.sync_info
                if si is not None and si.on_wait and len(si.on_wait) > 1:
                    waits = list(si.on_wait)
                    nops = []
                    for j, w in enumerate(waits[:-1]):
                        nop = mybir.InstNoOp(
                            name=f"{inst.name}-wsplit{j}", ins=[], outs=[]
                        )
                        nop.engine = inst.engine
                        nop.sync_info = mybir.SyncInfo(on_wait=[w], on_update=[])
                        nc.register_instruction(nop)
                        nops.append(nop)
                    inst.sync_info = mybir.SyncInfo(
                        on_wait=[waits[-1]], on_update=list(si.on_update)
                    )
                    insts[i:i] = nops
                    i += len(nops)
                i += 1
    return


def _get_nc() -> bass.Bass:
    global _NC_CACHE
    if _NC_CACHE is None:
        _NC_CACHE = _build()
    return _NC_CACHE


def _make_in_maps(x, Wp, bp, Wv):
    x = np.ascontiguousarray(np.asarray(x, dtype=np.float32))
    Wp = np.asarray(Wp, dtype=np.float32)
    bp = np.asarray(bp, dtype=np.float32)
    Wv = np.asarray(Wv, dtype=np.float32)

    # fold the tiny weights (O(D^2) host prep)
    p = np.arange(S, dtype=np.float32)
    pos = p @ Wp.T + bp                       # (S,)
    wv = Wv.sum(axis=0)                       # (D,) column sums
    bias8 = (pos * wv.sum()).astype(np.float32)
    bias_rpp = np.tile(bias8, RPP // S)       # (RPP,) pattern per in-partition row
    wb_row = np.concatenate([wv, bias_rpp])   # (D + RPP,)
    wb = np.ascontiguousarray(np.broadcast_to(wb_row, (P, D + RPP)), dtype=np.float32)
    import ml_dtypes

    wvh = np.ascontiguousarray(
        np.broadcast_to(wv.astype(ml_dtypes.bfloat16), (P, D))
    )

    xf = x.reshape(B * S * D)
    in_maps = []
    for i in range(N_CORES):
        shard = xf[i * ROWS * D : (i + 1) * ROWS * D].reshape(P, FREE)
        in_maps.append({"x": shard, "wb": wb, "wvh": wvh})
    return in_maps


def _run(x, Wp, bp, Wv, trace=False, **spmd_kwargs):
    nc = _get_nc()
    in_maps = _make_in_maps(x, Wp, bp, Wv)
    res = run_bass_kernel_spmd(
        nc, in_maps, list(range(N_CORES)), trace=trace, **spmd_kwargs
    )
    parts = [
        np.asarray(res.results[i]["out"]).astype(np.float32).reshape(BPC, S, D)
        for i in range(N_CORES)
    ]
    return np.concatenate(parts, axis=0), res


def kernel(x, Wp, bp, Wv, Wk, Wq) -> np.ndarray:
    out, _ = _run(x, Wp, bp, Wv)
    return out


# revision 43
# speedup vs baseline: 1.0289x; 1.0289x over previous
"""Trainium2 Bass kernel for nn_Attention_Temp_1468878815458.

Math: the reference computes
    pos   = arange(S) @ Wp.T + bp                       # (S,)
    embed = x.squeeze(1) + pos[:, None]                 # (B,S,D)
    v/k/q = embed @ {Wv,Wk,Wq}.T
    scores[b,x,y]  = (sum_q queries[b,q,x]) * (sum_k keys[b,k,y])
    attention      = softmax(scores, axis=1)            # over x
    out[b,v,y]     = sum_x attention[b,x,y] * sum_n values[b,v,n]

Since softmax normalizes over axis=1 and is then *summed* over axis=1,
sum_x attention[b,x,y] == 1 exactly.  Therefore
    out[b,s,y] = sum_n values[b,s,n]
               = (x[b,0,s,:] + pos[s]) . wv      for every y,
where wv[d] = sum_n Wv[n,d].  The kernel streams x once, computes the
per-row weighted sum with wv, adds the per-s bias pos[s]*sum(wv), and
broadcasts the scalar across the last dim.

Sharding: pure data parallel over batch, 1024 batches per core.  Each
core's shard is viewed as (128 partitions, 6144 f32): partition p holds
64 consecutive rows (8 batches x 8 seq) contiguously -> fully
contiguous DMA in AND out (24KB runs/partition).
"""

import numpy as np

import concourse.bass as bass
import concourse.mybir as mybir
from concourse.bass import broadcast_tensor_aps
from concourse.bass_utils import run_bass_kernel_spmd
from concourse.tile import TileContext

N_CORES = 8
B, S, D = 8192, 8, 96
BPC = B // N_CORES          # 1024 batches per core
ROWS = BPC * S              # 8192 rows of length D per core
P = 128                     # SBUF partitions
FREE = ROWS * D // P        # 6144 f32 per partition
RPP = ROWS // P             # 64 rows per partition
# pipeline chunk sizes in rows-per-partition: small chunks at the start
# (compute starts sooner) and end (shorter drain tail), big in the middle
# (fewer DMA triggers / per-op overheads).  GP_MUL marks chunks whose
# multiply runs on GPSIMD (otherwise idle) to shorten the DVE span.
CHUNK_ROWS = [8, 8, 12, 12, 12, 8, 3, 1]
# last chunks run their whole tail (bias, broadcast) on DVE to avoid
# cross-engine hops after the final reduce
DVE_TAIL = 2
assert sum(CHUNK_ROWS) == RPP
NCH = len(CHUNK_ROWS)

_NC_CACHE = None


def _build() -> bass.Bass:
    # seq codegen lowers multi-wait sync (e.g. the kernel-tail drain) to
    # sequencer commands; this walrus build allows only 1 wait per inst
    nc = bass.Bass(use_seq_codegen=True, enable_partition_id=False)
    x = nc.declare_dram_parameter("x", [P, FREE], mybir.dt.float32, isOutput=False)
    # combined constants: [:, :D] = wv replicated, [:, D:D+RPP] = per-row bias
    wb = nc.declare_dram_parameter("wb", [P, D + RPP], mybir.dt.float32, isOutput=False)
    # wv again, pre-cast to bf16 (the x stream is cast f32->bf16 in-DMA,
    # which makes the DVE multiply eligible for the 2x perf mode)
    wvh = nc.declare_dram_parameter("wvh", [P, D], mybir.dt.bfloat16, isOutput=False)
    # bf16 output halves the out-stream HBM bytes; host upcasts to f32.
    # rowdot values are O(10); bf16 keeps rel err ~4e-3, well under budget
    out = nc.declare_dram_parameter("out", [P, FREE], mybir.dt.bfloat16, isOutput=True)

    with TileContext(nc) as tc:
        with (
            tc.tile_pool(name="const", bufs=1) as cpool,
            # unique tag per chunk -> each tile gets its own slot: no slot
            # reuse, no WAR waits
            tc.tile_pool(name="xp", bufs=1) as xpool,
            tc.tile_pool(name="pp", bufs=4) as ppool,
            tc.tile_pool(name="op", bufs=1) as opool,
            tc.tile_pool(name="rp", bufs=1) as rpool,
        ):
            wb_sb = cpool.tile([P, D + RPP], mybir.dt.float32)
            # issued first on the sync ring: completes long before any
            # consumer; its waits are absorbed by the NOP-split pass
            nc.sync.dma_start(out=wb_sb[:], in_=wb[:])
            bias_sb = wb_sb[:, D : D + RPP]
            wvh_sb = cpool.tile([P, D], mybir.dt.bfloat16)
            nc.sync.dma_start(out=wvh_sb[:], in_=wvh[:])

            r0 = 0
            ot = None
            ot_r0 = 0
            ot_fill = 0
            for c, chr_ in enumerate(CHUNK_ROWS):
                chf = chr_ * D
                f0 = r0 * D
                xt = xpool.tile([P, chf], mybir.dt.bfloat16, tag=f"xt{c}")
                # SWDGE: casts f32 -> bf16 in the DMA datapath
                nc.gpsimd.dma_start(out=xt[:], in_=x[:, f0 : f0 + chf])

                x3 = xt[:].rearrange("p (r d) -> p r d", d=D)
                wv3 = wvh_sb[:].rearrange("p (r d) -> p r d", r=1)
                _, wv3b = broadcast_tensor_aps(x3, wv3)
                pt = ppool.tile([P, chf], mybir.dt.bfloat16, tag="pt")
                p3 = pt[:, :chf].rearrange("p (r d) -> p r d", d=D)
                nc.vector.tensor_tensor(
                    out=p3, in0=x3, in1=wv3b, op=mybir.AluOpType.mult
                )
                # fold the 96-wide rows to 48 with a 2x-mode bf16 add, then
                # reduce 48 -> 1: ~35% less DVE time than reducing 96 wide
                h = D // 2
                lo = p3[:, :, :h]
                hi = p3[:, :, h:]
                nc.vector.tensor_tensor(
                    out=lo, in0=lo, in1=hi, op=mybir.AluOpType.add
                )

                rd = rpool.tile([P, chr_], mybir.dt.float32, tag=f"rd{c}")
                nc.vector.reduce_sum(out=rd[:], in_=lo, axis=mybir.AxisListType.X)
                tail = c >= NCH - DVE_TAIL
                # bias add on the otherwise-idle GPSIMD engine (DVE at tail)
                add_eng = nc.vector if tail else nc.gpsimd
                add_eng.tensor_add(
                    out=rd[:], in0=rd[:], in1=bias_sb[:, r0 : r0 + chr_]
                )

                # out tiles hold two consecutive chunks -> half the out-DMA
                # triggers on the ACT ring
                if ot is None:
                    pair_rows = chr_ + (CHUNK_ROWS[c + 1] if c + 1 < NCH else 0)
                    pair_free = pair_rows * D
                    ot = opool.tile([P, pair_free], mybir.dt.bfloat16, tag=f"ot{c}")
                    ot_r0 = r0
                    ot_fill = 0
                ot3 = ot[:, ot_fill : ot_fill + chf].rearrange(
                    "p (r d) -> p r d", d=D
                )
                rd3 = rd[:].rearrange("p (r d) -> p r d", d=1)
                _, rd3b = broadcast_tensor_aps(ot3, rd3)
                if tail:
                    nc.vector.tensor_copy(out=ot3, in_=rd3b)
                else:
                    nc.scalar.copy(out=ot3, in_=rd3b)
                ot_fill += chf
                r0 += chr_

                if ot_fill == pair_free or c + 1 == NCH:
                    # ACT ring: FIFO-clean (each out's broadcast precedes it
                    # in the ACT queue, so the trigger never stalls the ring)
                    nc.scalar.dma_start(
                        out=out[:, ot_r0 * D : ot_r0 * D + ot_fill],
                        in_=ot[:, :ot_fill],
                    )
                    ot = None
    _strip_unused_const_memsets(nc)
    _split_multi_waits(nc)
    return nc


def _strip_unused_const_memsets(nc: bass.Bass) -> None:
    """Bass unconditionally memsets 4 const SBUF tensors on GPSIMD in the
    preamble (~3us on the init-barrier critical path).  This kernel never
    reads them; drop the memsets.  The init all-engine barrier that
    followed them is also dead once they're gone: engines are independent
    until the Tile-emitted semaphores in the body, and NRT guarantees a
    clean sem state at NEFF start."""
    for f in nc.m.functions:
        for bb in f.blocks:
            if bb.name != "main":
                continue
            keep = []
            for inst in bb.instructions:
                if isinstance(
                    inst, mybir.InstMemset | mybir.InstDrain | mybir.InstEventSemaphore
                ):
                    continue
                keep.append(inst)
            if len(keep) != len(bb.instructions):
                bb.instructions[:] = keep


def _split_multi_waits(nc: bass.Bass) -> None:
    """Walrus (this build) allows only one sync wait per instruction.

    Tile's kernel-tail drain merges waits on every DMA lane + engine sem
    into one instruction; split the extras onto same-engine NOPs placed
    immediately before it.
    """
    for f in nc.m.functions:
        for bb in f.blocks:
            insts = bb.instructions
            i = 0
            while i < len(insts):
                inst = insts[i]
                si = inst.sync_info
                if si is not None and si.on_wait and len(si.on_wait) > 1:
                    waits = list(si.on_wait)
                    nops = []
                    for j, w in enumerate(waits[:-1]):
                        nop = mybir.InstNoOp(
                            name=f"{inst.name}-wsplit{j}", ins=[], outs=[]
                        )
                        nop.engine = inst.engine
                        nop.sync_info = mybir.SyncInfo(on_wait=[w], on_update=[])
                        nc.register_instruction(nop)
                        nops.append(nop)
                    inst.sync_info = mybir.SyncInfo(
                        on_wait=[waits[-1]], on_update=list(si.on_update)
                    )
                    insts[i:i] = nops
                    i += len(nops)
                i += 1
    return


def _get_nc() -> bass.Bass:
    global _NC_CACHE
    if _NC_CACHE is None:
        _NC_CACHE = _build()
    return _NC_CACHE


def _make_in_maps(x, Wp, bp, Wv):
    x = np.ascontiguousarray(np.asarray(x, dtype=np.float32))
    Wp = np.asarray(Wp, dtype=np.float32)
    bp = np.asarray(bp, dtype=np.float32)
    Wv = np.asarray(Wv, dtype=np.float32)

    # fold the tiny weights (O(D^2) host prep)
    p = np.arange(S, dtype=np.float32)
    pos = p @ Wp.T + bp                       # (S,)
    wv = Wv.sum(axis=0)                       # (D,) column sums
    bias8 = (pos * wv.sum()).astype(np.float32)
    bias_rpp = np.tile(bias8, RPP // S)       # (RPP,) pattern per in-partition row
    wb_row = np.concatenate([wv, bias_rpp])   # (D + RPP,)
    wb = np.ascontiguousarray(np.broadcast_to(wb_row, (P, D + RPP)), dtype=np.float32)
    import ml_dtypes

    wvh = np.ascontiguousarray(
        np.broadcast_to(wv.astype(ml_dtypes.bfloat16), (P, D))
    )

    xf = x.reshape(B * S * D)
    in_maps = []
    for i in range(N_CORES):
        shard = xf[i * ROWS * D : (i + 1) * ROWS * D].reshape(P, FREE)
        in_maps.append({"x": shard, "wb": wb, "wvh": wvh})
    return in_maps


def _run(x, Wp, bp, Wv, trace=False, **spmd_kwargs):
    nc = _get_nc()
    in_maps = _make_in_maps(x, Wp, bp, Wv)
    res = run_bass_kernel_spmd(
        nc, in_maps, list(range(N_CORES)), trace=trace, **spmd_kwargs
    )
    parts = [
        np.asarray(res.results[i]["out"]).astype(np.float32).reshape(BPC, S, D)
        for i in range(N_CORES)
    ]
    return np.concatenate(parts, axis=0), res


def kernel(x, Wp, bp, Wv, Wk, Wq) -> np.ndarray:
    out, _ = _run(x, Wp, bp, Wv)
    return out


# revision 45
# speedup vs baseline: 1.0429x; 1.0136x over previous
"""Trainium2 Bass kernel for nn_Attention_Temp_1468878815458.

Math: the reference computes
    pos   = arange(S) @ Wp.T + bp                       # (S,)
    embed = x.squeeze(1) + pos[:, None]                 # (B,S,D)
    v/k/q = embed @ {Wv,Wk,Wq}.T
    scores[b,x,y]  = (sum_q queries[b,q,x]) * (sum_k keys[b,k,y])
    attention      = softmax(scores, axis=1)            # over x
    out[b,v,y]     = sum_x attention[b,x,y] * sum_n values[b,v,n]

Since softmax normalizes over axis=1 and is then *summed* over axis=1,
sum_x attention[b,x,y] == 1 exactly.  Therefore
    out[b,s,y] = sum_n values[b,s,n]
               = (x[b,0,s,:] + pos[s]) . wv      for every y,
where wv[d] = sum_n Wv[n,d].  The kernel streams x once, computes the
per-row weighted sum with wv, adds the per-s bias pos[s]*sum(wv), and
broadcasts the scalar across the last dim.

Sharding: pure data parallel over batch, 1024 batches per core.  Each
core's shard is viewed as (128 partitions, 6144 f32): partition p holds
64 consecutive rows (8 batches x 8 seq) contiguously -> fully
contiguous DMA in AND out (24KB runs/partition).
"""

import numpy as np

import concourse.bass as bass
import concourse.mybir as mybir
from concourse.bass import broadcast_tensor_aps
from concourse.bass_utils import run_bass_kernel_spmd
from concourse.tile import TileContext

N_CORES = 8
B, S, D = 8192, 8, 96
BPC = B // N_CORES          # 1024 batches per core
ROWS = BPC * S              # 8192 rows of length D per core
P = 128                     # SBUF partitions
FREE = ROWS * D // P        # 6144 f32 per partition
RPP = ROWS // P             # 64 rows per partition
# pipeline chunk sizes in rows-per-partition: small chunks at the start
# (compute starts sooner) and end (shorter drain tail), big in the middle
# (fewer DMA triggers / per-op overheads).  GP_MUL marks chunks whose
# multiply runs on GPSIMD (otherwise idle) to shorten the DVE span.
CHUNK_ROWS = [8, 8, 12, 12, 12, 8, 3, 1]
# last chunks run their whole tail (bias, broadcast) on DVE to avoid
# cross-engine hops after the final reduce
DVE_TAIL = 2
assert sum(CHUNK_ROWS) == RPP
NCH = len(CHUNK_ROWS)

_NC_CACHE = None


def _build() -> bass.Bass:
    # seq codegen lowers multi-wait sync (e.g. the kernel-tail drain) to
    # sequencer commands; this walrus build allows only 1 wait per inst
    nc = bass.Bass(use_seq_codegen=True, enable_partition_id=False)
    x = nc.declare_dram_parameter("x", [P, FREE], mybir.dt.float32, isOutput=False)
    # combined constants: [:, :D] = wv replicated, [:, D:D+RPP] = per-row bias
    wb = nc.declare_dram_parameter("wb", [P, D + RPP], mybir.dt.float32, isOutput=False)
    # wv again, pre-cast to bf16 (the x stream is cast f32->bf16 in-DMA,
    # which makes the DVE multiply eligible for the 2x perf mode)
    wvh = nc.declare_dram_parameter("wvh", [P, D], mybir.dt.bfloat16, isOutput=False)
    # bf16 output halves the out-stream HBM bytes; host upcasts to f32.
    # rowdot values are O(10); bf16 keeps rel err ~4e-3, well under budget
    out = nc.declare_dram_parameter("out", [P, FREE], mybir.dt.bfloat16, isOutput=True)

    with TileContext(nc) as tc:
        with (
            tc.tile_pool(name="const", bufs=1) as cpool,
            # unique tag per chunk -> each tile gets its own slot: no slot
            # reuse, no WAR waits
            tc.tile_pool(name="xp", bufs=1) as xpool,
            tc.tile_pool(name="pp", bufs=4) as ppool,
            tc.tile_pool(name="op", bufs=1) as opool,
            tc.tile_pool(name="rp", bufs=1) as rpool,
        ):
            wb_sb = cpool.tile([P, D + RPP], mybir.dt.float32)
            # issued first on the sync ring: completes long before any
            # consumer; its waits are absorbed by the NOP-split pass
            nc.sync.dma_start(out=wb_sb[:], in_=wb[:])
            bias_sb = wb_sb[:, D : D + RPP]
            wvh_sb = cpool.tile([P, D], mybir.dt.bfloat16)
            nc.sync.dma_start(out=wvh_sb[:], in_=wvh[:])

            r0 = 0
            ot = None
            ot_r0 = 0
            ot_fill = 0
            pending_outs = []
            for c, chr_ in enumerate(CHUNK_ROWS):
                chf = chr_ * D
                f0 = r0 * D
                xt = xpool.tile([P, chf], mybir.dt.bfloat16, tag=f"xt{c}")
                # SWDGE: casts f32 -> bf16 in the DMA datapath
                nc.gpsimd.dma_start(out=xt[:], in_=x[:, f0 : f0 + chf])

                x3 = xt[:].rearrange("p (r d) -> p r d", d=D)
                wv3 = wvh_sb[:].rearrange("p (r d) -> p r d", r=1)
                _, wv3b = broadcast_tensor_aps(x3, wv3)
                pt = ppool.tile([P, chf], mybir.dt.bfloat16, tag="pt")
                p3 = pt[:, :chf].rearrange("p (r d) -> p r d", d=D)
                nc.vector.tensor_tensor(
                    out=p3, in0=x3, in1=wv3b, op=mybir.AluOpType.mult
                )
                # fold the 96-wide rows to 48 with a 2x-mode bf16 add, then
                # reduce 48 -> 1: ~35% less DVE time than reducing 96 wide
                h = D // 2
                lo = p3[:, :, :h]
                hi = p3[:, :, h:]
                nc.vector.tensor_tensor(
                    out=lo, in0=lo, in1=hi, op=mybir.AluOpType.add
                )

                rd = rpool.tile([P, chr_], mybir.dt.float32, tag=f"rd{c}")
                nc.vector.reduce_sum(out=rd[:], in_=lo, axis=mybir.AxisListType.X)
                tail = c >= NCH - DVE_TAIL
                # bias add on the otherwise-idle GPSIMD engine (DVE at tail)
                add_eng = nc.vector if tail else nc.gpsimd
                add_eng.tensor_add(
                    out=rd[:], in0=rd[:], in1=bias_sb[:, r0 : r0 + chr_]
                )

                # out tiles hold two consecutive chunks -> half the out-DMA
                # triggers on the ACT ring
                if ot is None:
                    pair_rows = chr_ + (CHUNK_ROWS[c + 1] if c + 1 < NCH else 0)
                    pair_free = pair_rows * D
                    ot = opool.tile([P, pair_free], mybir.dt.bfloat16, tag=f"ot{c}")
                    ot_r0 = r0
                    ot_fill = 0
                ot3 = ot[:, ot_fill : ot_fill + chf].rearrange(
                    "p (r d) -> p r d", d=D
                )
                rd3 = rd[:].rearrange("p (r d) -> p r d", d=1)
                _, rd3b = broadcast_tensor_aps(ot3, rd3)
                if tail:
                    nc.vector.tensor_copy(out=ot3, in_=rd3b)
                else:
                    nc.scalar.copy(out=ot3, in_=rd3b)
                ot_fill += chf
                r0 += chr_

                if ot_fill == pair_free or c + 1 == NCH:
                    # deferred to the end of the build: the SP ring is FIFO,
                    # so out-triggers must sit behind ALL in-triggers or the
                    # in-stream stalls behind a waiting out-trigger
                    pending_outs.append(
                        (out[:, ot_r0 * D : ot_r0 * D + ot_fill], ot[:, :ot_fill])
                    )
                    ot = None
            for dst, src in pending_outs:
                nc.sync.dma_start(out=dst, in_=src)
    _strip_unused_const_memsets(nc)
    _split_multi_waits(nc)
    return nc


def _strip_unused_const_memsets(nc: bass.Bass) -> None:
    """Bass unconditionally memsets 4 const SBUF tensors on GPSIMD in the
    preamble (~3us on the init-barrier critical path).  This kernel never
    reads them; drop the memsets.  The init all-engine barrier that
    followed them is also dead once they're gone: engines are independent
    until the Tile-emitted semaphores in the body, and NRT guarantees a
    clean sem state at NEFF start."""
    for f in nc.m.functions:
        for bb in f.blocks:
            if bb.name != "main":
                continue
            keep = []
            for inst in bb.instructions:
                if isinstance(
                    inst, mybir.InstMemset | mybir.InstDrain | mybir.InstEventSemaphore
                ):
                    continue
                keep.append(inst)
            if len(keep) != len(bb.instructions):
                bb.instructions[:] = keep


def _split_multi_waits(nc: bass.Bass) -> None:
    """Walrus (this build) allows only one sync wait per instruction.

    Tile's kernel-tail drain merges waits on every DMA lane + engine sem
    into one instruction; split the extras onto same-engine NOPs placed
    immediately before it.
    """
    for f in nc.m.functions:
        for bb in f.blocks:
            insts = bb.instructions
            i = 0
            while i < len(insts):
                inst = insts[i]
                si = inst.sync_info
                if si is not None and si.on_wait and len(si.on_wait) > 1:
                    waits = list(si.on_wait)
                    nops = []
                    for j, w in enumerate(waits[:-1]):
                        nop = mybir.InstNoOp(
                            name=f"{inst.name}-wsplit{j}", ins=[], outs=[]
                        )
                        nop.engine = inst.engine
                        nop.sync_info = mybir.SyncInfo(on_wait=[w], on_update=[])
                        nc.register_instruction(nop)
                        nops.append(nop)
                    inst.sync_info = mybir.SyncInfo(
                        on_wait=[waits[-1]], on_update=list(si.on_update)
                    )
                    insts[i:i] = nops
                    i += len(nops)
                i += 1
    return


def _get_nc() -> bass.Bass:
    global _NC_CACHE
    if _NC_CACHE is None:
        _NC_CACHE = _build()
    return _NC_CACHE


def _make_in_maps(x, Wp, bp, Wv):
    x = np.ascontiguousarray(np.asarray(x, dtype=np.float32))
    Wp = np.asarray(Wp, dtype=np.float32)
    bp = np.asarray(bp, dtype=np.float32)
    Wv = np.asarray(Wv, dtype=np.float32)

    # fold the tiny weights (O(D^2) host prep)
    p = np.arange(S, dtype=np.float32)
    pos = p @ Wp.T + bp                       # (S,)
    wv = Wv.sum(axis=0)                       # (D,) column sums
    bias8 = (pos * wv.sum()).astype(np.float32)
    bias_rpp = np.tile(bias8, RPP // S)       # (RPP,) pattern per in-partition row
    wb_row = np.concatenate([wv, bias_rpp])   # (D + RPP,)
    wb = np.ascontiguousarray(np.broadcast_to(wb_row, (P, D + RPP)), dtype=np.float32)
    import ml_dtypes

    wvh = np.ascontiguousarray(
        np.broadcast_to(wv.astype(ml_dtypes.bfloat16), (P, D))
    )

    xf = x.reshape(B * S * D)
    in_maps = []
    for i in range(N_CORES):
        shard = xf[i * ROWS * D : (i + 1) * ROWS * D].reshape(P, FREE)
        in_maps.append({"x": shard, "wb": wb, "wvh": wvh})
    return in_maps


def _run(x, Wp, bp, Wv, trace=False, **spmd_kwargs):
    nc = _get_nc()
    in_maps = _make_in_maps(x, Wp, bp, Wv)
    res = run_bass_kernel_spmd(
        nc, in_maps, list(range(N_CORES)), trace=trace, **spmd_kwargs
    )
    parts = [
        np.asarray(res.results[i]["out"]).astype(np.float32).reshape(BPC, S, D)
        for i in range(N_CORES)
    ]
    return np.concatenate(parts, axis=0), res


def kernel(x, Wp, bp, Wv, Wk, Wq) -> np.ndarray:
    out, _ = _run(x, Wp, bp, Wv)
    return out
